# revision 20
# baseline (speedup 1.0000x reference)
"""Trainium2 Bass kernel: 2-layer GCN (PyG-style GCNConv x2) on 8 NeuronCores.

Strategy (v2, src-sharded):
  - Nodes sharded contiguously across 8 cores (12500 rows each); edges are
    processed on the core that OWNS THE SOURCE node, so the per-layer gather
    reads only the core's own 12500-row table (kept in both SBUF and DRAM,
    bf16).
  - Per layer: dense h' = (x @ W) * dinv[src] on the owning core; the edge
    stream (sorted by dst) is gathered in 2048-slot super-blocks split
    between DRAM-source dma_gather (1024 idx, slot-major) and SBUF-source
    transpose dma_gather (2x512 idx, feat-major + PE transpose), spread over
    4 SWDGE queues so descriptor generation and transfers overlap.
  - Scatter-add via one-hot matmuls (bf16) into PSUM accumulators, one
    [128, 2, 128] accumulator per 256-dst-node "pair" segment; partial sums
    for all 100K dst nodes are evicted bf16 and combined with a
    ReduceScatter(add), which lands each core's own 12500 rows.
  - dinv[dst] scaling + bias are applied after the ReduceScatter on the
    owner core; layer 2 repeats the pipeline on the layer-1 output.
"""

import os
import sys

for _p in ("/opt/trn_rl_repo",):
    if _p not in sys.path:
        sys.path.append(_p)

import numpy as np

import concourse.bacc as bacc
import concourse.mybir as mybir
import concourse.tile as tile
from concourse.bass_utils import run_bass_kernel_spmd

F32 = mybir.dt.float32
BF16 = mybir.dt.bfloat16
I16 = mybir.dt.int16
AF = mybir.ActivationFunctionType
ALU = mybir.AluOpType

N_NODES = 100000
D = 128
NCORES = 8
NS = N_NODES // NCORES           # 12500
NTILE = -(-NS // 128)            # 98
NPAIR = -(-NS // 256)            # 49
NPID = NCORES * NPAIR            # 392
SB = 2048                        # slots per super-block
SBCH = SB // 128                 # 16 chunks per super-block


def _bf16(a):
    try:
        import ml_dtypes
        return np.asarray(a, dtype=ml_dtypes.bfloat16)
    except ImportError:
        import jax.numpy as jnp
        return np.asarray(jnp.asarray(a, dtype=jnp.bfloat16))


class Plan:
    """Core-uniform gather/matmul schedule derived from the edge index."""

    def __init__(self, n_nodes, edge_index):
        assert n_nodes == N_NODES
        e = np.asarray(edge_index)
        # self-loops are excluded from the gather stream: their contribution
        # dinv[i]*h'[i] is local to the owner and is added in post().
        src = np.asarray(e[0], np.int64)
        dst = np.asarray(e[1], np.int64)
        deg = (np.bincount(dst, minlength=n_nodes) + 1).astype(np.float32)
        self.dinv = deg ** -0.5

        owner = src // NS
        order = np.argsort(owner * n_nodes + dst, kind="stable")
        srcl = (src - owner * NS)[order].astype(np.int16)
        dsts = dst[order]
        own = owner[order].astype(np.int64)
        r = dsts % NS
        pid = (dsts // NS) * NPAIR + r // 256
        sub = (r // 128) % 2
        doffv = (r % 128).astype(np.float32)

        E = len(srcl)
        cnt = np.bincount(own * NPID + pid, minlength=NCORES * NPID
                          ).reshape(NCORES, NPID)
        CT = np.maximum(-(-cnt.max(axis=0) // 128), 1)
        self.CT = CT
        CTmax = int(CT.max())
        sbase = np.zeros(NPID + 1, np.int64)
        np.cumsum(CT * 128, out=sbase[1:])
        raw_slots = int(sbase[-1])
        self.nsb = -(-raw_slots // SB)
        self.n_slots = self.nsb * SB
        nch = self.n_slots // 128

        segkey = own * NPID + pid
        changed = np.r_[True, segkey[1:] != segkey[:-1]]
        seg_first = np.flatnonzero(changed)
        rank = np.arange(E) - seg_first[np.cumsum(changed) - 1]
        jch = rank // 128
        sic = rank % 128
        abs_slot = sbase[pid] + rank

        self._idx_stream = np.zeros((NCORES, self.n_slots), np.int16)
        self._idx_stream[own, abs_slot] = srcl

        ekey = (pid * CTmax + jch) * 2 + sub
        emit = np.bincount(ekey, minlength=NPID * CTmax * 2) > 0
        emit = emit.reshape(NPID, CTmax, 2)
        jmask = np.arange(CTmax)[None, :] < CT[:, None]
        emit &= jmask[:, :, None]
        # every pair always has both subtiles (self-loops); ensure at least
        # one column per (pid, s) so every acc region is written
        for s in (0, 1):
            none = ~emit[:, :, s].any(axis=1)
            emit[none, 0, s] = True
        col_of = np.full((NPID, CTmax, 2), -1, np.int64)
        ncols = int(emit.sum())
        # column ids in (pid, j, s) lexicographic order == stream order
        flat = emit.reshape(-1)
        col_of.reshape(-1)[flat] = np.arange(ncols)
        self.ncols = ncols

        doff_t = np.full((NCORES, 128, ncols), -1.0, np.float32)
        colid = col_of[pid, jch, sub]
        assert (colid >= 0).all()
        doff_t[own, sic, colid] = doffv
        self._doff = doff_t

        pid_of_chunk = np.repeat(np.arange(NPID), CT)
        j_of_chunk = np.concatenate([np.arange(c) for c in CT])
        first_col = np.full((NPID, 2), -1, np.int64)
        last_col = np.full((NPID, 2), -1, np.int64)
        for s in (0, 1):
            for p in range(NPID):
                js = np.flatnonzero(emit[p, :, s])
                first_col[p, s] = col_of[p, js[0], s]
                last_col[p, s] = col_of[p, js[-1], s]
        self.raw_chunks = len(pid_of_chunk)
        self.mm_sched = [[] for _ in range(nch)]
        for k in range(self.raw_chunks):
            p, j = int(pid_of_chunk[k]), int(j_of_chunk[k])
            for s in (0, 1):
                c = int(col_of[p, j, s])
                if c >= 0:
                    self.mm_sched[k].append(
                        (c, p, s, c == first_col[p, s], c == last_col[p, s]))
        self.ev_after = [[] for _ in range(nch)]
        for p in range(NPID):
            k_last = int(sbase[p] // 128 + CT[p] - 1)
            self.ev_after[k_last].append(p)

        self.sb_cols = []
        for sbi in range(self.nsb):
            k0 = sbi * SBCH
            k1 = min((sbi + 1) * SBCH, self.raw_chunks)
            cols = [c for k in range(k0, k1) for (c, *_r) in self.mm_sched[k]]
            self.sb_cols.append((min(cols), max(cols) + 1) if cols else (0, 0))
        self.max_sb_cols = max(c1 - c0 for c0, c1 in self.sb_cols)

        self.call_live = []
        for sbi in range(self.nsb):
            base = sbi * SB
            self.call_live.append((base < raw_slots,
                                   base + 1024 < raw_slots,
                                   base + 1536 < raw_slots))
        self.raw_slots = raw_slots
        self.icols = self.nsb * 128

    def core_inputs(self, c):
        st = self._idx_stream[c].reshape(self.nsb, SB)
        idx = np.zeros((16, self.nsb, 128), np.int16)
        idx[:, :, 0:64] = st[:, :1024].reshape(self.nsb, 64, 16
                                               ).transpose(2, 0, 1)
        idx[:, :, 64:96] = st[:, 1024:1536].reshape(self.nsb, 32, 16
                                                    ).transpose(2, 0, 1)
        idx[:, :, 96:128] = st[:, 1536:2048].reshape(self.nsb, 32, 16
                                                     ).transpose(2, 0, 1)
        idx = idx.reshape(16, self.icols)
        return np.tile(idx, (8, 1)), self._doff[c]


def _build(plan):
    nc = bacc.Bacc("TRN2", target_bir_lowering=False, debug=False,
                   num_devices=NCORES, num_swdge_queues=4)
    xT_d = nc.dram_tensor("xT", [D, NS], BF16, kind="ExternalInput").ap()
    wts_d = nc.dram_tensor("wts", [D, 2, D], BF16, kind="ExternalInput").ap()
    bias_d = nc.dram_tensor("bias", [D, 2, D], F32, kind="ExternalInput").ap()
    dv_d = nc.dram_tensor("dinv_c", [D, NTILE], F32, kind="ExternalInput").ap()
    iota_d = nc.dram_tensor("iota", [D, D], F32, kind="ExternalInput").ap()
    idn_d = nc.dram_tensor("identb", [D, D], BF16, kind="ExternalInput").ap()
    idx_d = nc.dram_tensor("idx", [D, plan.icols], I16,
                           kind="ExternalInput").ap()
    doff_d = nc.dram_tensor("doff", [D, plan.ncols], F32,
                            kind="ExternalInput").ap()
    out_d = nc.dram_tensor("out", [NS, D], F32, kind="ExternalOutput").ap()

    dbg = os.environ.get("GCN_DEBUG", "")
    def _k(nm):
        return "ExternalOutput" if nm in dbg.split(",") else "Internal"
    tbl_d = [nc.dram_tensor(f"tbl{i}", [NTILE * 128, D], BF16,
                            kind=_k(f"tbl{i}")).ap() for i in range(2)]
    part_d = [nc.dram_tensor(f"part{i}", [N_NODES, D], BF16,
                             kind=_k(f"part{i}")).ap() for i in range(2)]
    shard_d = [nc.dram_tensor(f"shard{i}", [NS, D], BF16,
                              kind=_k(f"shard{i}")).ap() for i in range(2)]

    with tile.TileContext(nc) as tc:
        with (
            tc.tile_pool(name="const", bufs=1) as cpool,
            tc.tile_pool(name="stg", bufs=6) as stgpool,
            tc.tile_pool(name="gg", bufs=6) as gpool,
            tc.tile_pool(name="tt", bufs=6) as tpool,
            tc.tile_pool(name="oh", bufs=4) as ohpool,
            tc.tile_pool(name="ev", bufs=6) as evpool,
            tc.tile_pool(name="ld", bufs=4) as ldpool,
            tc.tile_pool(name="acc", bufs=4, space="PSUM") as accpool,
            tc.tile_pool(name="tr", bufs=2, space="PSUM") as trpool,
            tc.tile_pool(name="pp", bufs=1, space="PSUM") as pppool,
        ):
            w_sb = cpool.tile([D, 2, D], BF16, tag="w")
            nc.sync.dma_start(out=w_sb[:], in_=wts_d[:])
            bias_sb = cpool.tile([D, 2, D], F32, tag="bias")
            nc.sync.dma_start(out=bias_sb[:], in_=bias_d[:])
            dv_sb = cpool.tile([D, NTILE], F32, tag="dv")
            nc.sync.dma_start(out=dv_sb[:], in_=dv_d[:])
            iota_sb = cpool.tile([D, D], F32, tag="iota")
            nc.sync.dma_start(out=iota_sb[:], in_=iota_d[:])
            idn_sb = cpool.tile([D, D], BF16, tag="idn")
            nc.sync.dma_start(out=idn_sb[:], in_=idn_d[:])
            idx_sb = cpool.tile([D, plan.icols], I16, tag="idx")
            nc.sync.dma_start(out=idx_sb[:], in_=idx_d[:])
            doff_sb = cpool.tile([D, plan.ncols], F32, tag="doff")
            nc.sync.dma_start(out=doff_sb[:], in_=doff_d[:])
            xT_sb = cpool.tile([D, NS], BF16, tag="xT")
            nc.sync.dma_start(out=xT_sb[:], in_=xT_d[:])
            xT2_sb = cpool.tile([D, NS], BF16, tag="xT2")
            tbl_sb = [cpool.tile([D, NTILE, D], BF16, tag=f"tblsb{i}",
                                 name=f"tblsb{i}")
                      for i in range(2)]

            def tw(t):
                return 128 if t < NTILE - 1 else NS - 128 * (NTILE - 1)

            def dense(li, src_sb):
                # last stripe is only partially covered by rows; zero it so
                # the full-tile SBUF gather source is fully initialized
                nc.vector.memset(tbl_sb[li][:, NTILE - 1, :], 0.0)
                for t in range(NTILE):
                    w = tw(t)
                    pdt = pppool.tile([128, D], F32, tag="pd")
                    nc.tensor.matmul(pdt[:w, :],
                                     lhsT=src_sb[:, t * 128:t * 128 + w],
                                     rhs=w_sb[:, li, :], start=True, stop=True)
                    nc.scalar.activation(tbl_sb[li][:w, t, :], pdt[:w, :],
                                         AF.Copy, scale=dv_sb[:w, t:t + 1])
                    nc.sync.dma_start(out=tbl_d[li][t * 128:(t + 1) * 128, :],
                                      in_=tbl_sb[li][:, t, :])

            qctr = [0]

            def sparse(li):
                acc_open = {}
                for sbi in range(plan.nsb):
                    dlive, s0live, s1live = plan.call_live[sbi]
                    ibase = sbi * 128
                    stg = None
                    tts = [None, None]
                    if dlive:
                        stg = stgpool.tile([128, 8, D], BF16, tag="stg")
                        nc.gpsimd.dma_gather(stg[:], tbl_d[li][:],
                                             idx_sb[:, ibase:ibase + 64],
                                             1024, 1024, D,
                                             queue_num=qctr[0] % 4)
                        qctr[0] += 1
                    for m, live in enumerate((s0live, s1live)):
                        if not live:
                            continue
                        g = gpool.tile([128, 1, 512], BF16, tag="g")
                        nc.gpsimd.dma_gather(
                            g[:], tbl_sb[li][:],
                            idx_sb[:, ibase + 64 + 32 * m:
                                   ibase + 96 + 32 * m],
                            512, 512, D, transpose=True,
                            sbuf_tokens_per_rank=128,
                            sbuf_free_dim_per_rank=2 * D,
                            sbuf_free_dim_pad_per_rank=0,
                            sbuf_byte_offset=0,
                            queue_num=qctr[0] % 4)
                        qctr[0] += 1
                        tp = trpool.tile([128, 4, D], BF16, tag="tp")
                        for cc in range(4):
                            nc.tensor.transpose(tp[:, cc, :],
                                                g[:, 0, cc * 128:
                                                  (cc + 1) * 128],
                                                idn_sb[:])
                        tb = tpool.tile([128, 4, D], BF16, tag="tb")
                        nc.vector.tensor_copy(tb[:], tp[:])
                        tts[m] = tb
                    c0, c1 = plan.sb_cols[sbi]
                    oh = None
                    if c1 > c0:
                        oh = ohpool.tile([128, plan.max_sb_cols, D], BF16,
                                         tag="oh")
                        nc.vector.scalar_tensor_tensor(
                            out=oh[:, :c1 - c0, :],
                            in0=doff_sb[:, c0:c1].unsqueeze(2)
                                .broadcast_to([128, c1 - c0, D]),
                            scalar=1.0,
                            in1=iota_sb[:].unsqueeze(1)
                                .broadcast_to([128, c1 - c0, D]),
                            op0=ALU.mult, op1=ALU.is_equal)
                    k1 = min((sbi + 1) * SBCH, plan.raw_chunks)
                    for k in range(sbi * SBCH, k1):
                        j = k - sbi * SBCH
                        if j < 8:
                            rhs = stg[:, j, :]
                        else:
                            rhs = tts[(j - 8) // 4][:, (j - 8) % 4, :]
                        for (c, p, s, st, sp) in plan.mm_sched[k]:
                            if (p, s) not in acc_open:
                                acc_open[(p, s)] = accpool.tile(
                                    [128, D], F32, tag="acc",
                                    name=f"a{li}_{p}_{s}")
                            nc.tensor.matmul(acc_open[(p, s)][:],
                                             lhsT=oh[:, c - c0, :], rhs=rhs,
                                             start=st, stop=sp)
                        for p in plan.ev_after[k]:
                            dc, pl = divmod(p, NPAIR)
                            r0 = dc * NS + pl * 256
                            for s in (0, 1):
                                acc = acc_open.pop((p, s))
                                w = min(128, NS - pl * 256 - s * 128)
                                if w <= 0:
                                    continue
                                ev = evpool.tile([128, D], BF16, tag="ev")
                                nc.scalar.activation(ev[:w, :], acc[:w, :],
                                                     AF.Copy)
                                nc.sync.dma_start(
                                    out=part_d[li][r0 + s * 128:
                                                   r0 + s * 128 + w, :],
                                    in_=ev[:w, :])
                assert not acc_open
                nc.gpsimd.collective_compute(
                    "ReduceScatter", ALU.add,
                    replica_groups=[list(range(NCORES))],
                    ins=[part_d[li].opt()], outs=[shard_d[li].opt()])

            def post(li):
                for t in range(NTILE):
                    w = tw(t)
                    sld = ldpool.tile([128, D], BF16, tag="sld")
                    nc.sync.dma_start(out=sld[:w, :],
                                      in_=shard_d[li][t * 128:t * 128 + w, :])
                    # add the self-loop term h'[i] (local table row), then
                    # scale by dinv[dst]
                    ys = ldpool.tile([128, D], F32, tag="ys")
                    nc.vector.tensor_tensor(out=ys[:w, :], in0=sld[:w, :],
                                            in1=tbl_sb[li][:w, t, :],
                                            op=ALU.add)
                    ya = ldpool.tile([128, D], F32, tag="ya")
                    nc.scalar.activation(ya[:w, :], ys[:w, :], AF.Copy,
                                         scale=dv_sb[:w, t:t + 1])
                    if li == 0:
                        yb = ldpool.tile([128, D], BF16, tag="yb")
                        nc.vector.tensor_tensor(out=yb[:w, :], in0=ya[:w, :],
                                                in1=bias_sb[:w, li, :],
                                                op=ALU.add)
                        ptr = pppool.tile([D, 128], BF16, tag="ptr")
                        nc.tensor.transpose(ptr[:, :w], yb[:w, :],
                                            idn_sb[:w, :w])
                        nc.vector.tensor_copy(xT2_sb[:, t * 128:t * 128 + w],
                                              ptr[:, :w])
                    else:
                        yb = ldpool.tile([128, D], F32, tag="ybf")
                        nc.vector.tensor_tensor(out=yb[:w, :], in0=ya[:w, :],
                                                in1=bias_sb[:w, li, :],
                                                op=ALU.add)
                        nc.sync.dma_start(out=out_d[t * 128:t * 128 + w, :],
                                          in_=yb[:w, :])

            dense(0, xT_sb)
            sparse(0)
            post(0)
            dense(1, xT2_sb)
            sparse(1)
            post(1)

    # The tile scheduler assigns DMASW completion lanes round-robin over the
    # *scheduled* SWDGE order, and each lane must stay bound to one SWDGE
    # queue. Rewrite queue_num := lane % 4 after scheduling so the binding is
    # consistent by construction.
    from concourse.tile_sem_assignment import PROC_NAME_TO_IDX
    lane_of = {PROC_NAME_TO_IDX[f"DMASW{i}"]: i for i in range(8)}
    for bb in nc.main_func.blocks:
        for inst in bb.instructions:
            proc = getattr(inst, "bass_scheduled_proc", None)
            if proc in lane_of and hasattr(inst, "queue_num"):
                inst.queue_num = lane_of[proc] % int(os.environ.get('GCN_NQ', '4'))

    nc.compile()
    return nc


def _install_ntff_hook():
    """antenv.axon_hooks is absent in this image; synthesize it and register
    the ctypes NTFF profile hook from the boot module."""
    import types
    if "antenv.axon_hooks" in sys.modules:
        return
    try:
        from trn_agent_boot.trn_boot import _ntff_profile_via_ctypes
        hook = _ntff_profile_via_ctypes("/opt/axon/libaxon_pjrt.so")
    except Exception as e:
        print(f"[kernel] ntff hook unavailable: {e}", flush=True)
        hook = None
    mod = types.ModuleType("antenv.axon_hooks")
    mod._hook = hook
    mod.set_axon_ntff_profile_hook = lambda h: setattr(mod, "_hook", h)
    mod.get_axon_ntff_profile_hook = lambda: mod._hook
    sys.modules["antenv.axon_hooks"] = mod
    import antenv
    antenv.axon_hooks = mod


def _run(plan, x, W1, b1, W2, b2, trace=False, stage="full"):
    import time
    if trace:
        _install_ntff_hook()
    t0 = time.time()
    nc = _build(plan)
    t1 = time.time()
    if os.environ.get("GCN_VERBOSE"):
        print(f"[kernel] build+compile: {t1 - t0:.1f}s", flush=True)

    wts = _bf16(np.stack([np.asarray(W1, np.float32),
                          np.asarray(W2, np.float32)], axis=1))
    bias = np.broadcast_to(
        np.stack([np.asarray(b1, np.float32), np.asarray(b2, np.float32)],
                 axis=0)[None, :, :], (D, 2, D)).copy()
    iota_t = np.tile(np.arange(D, dtype=np.float32), (D, 1))
    ident_b = _bf16(np.eye(D, dtype=np.float32))

    in_maps = []
    for c in range(NCORES):
        lo, hi = c * NS, (c + 1) * NS
        dcol = np.ones((NTILE, 128), dtype=np.float32)
        dcol.reshape(-1)[:NS] = plan.dinv[lo:hi]
        dcol = np.ascontiguousarray(dcol.T)
        idx, doff = plan.core_inputs(c)
        in_maps.append({
            "xT": _bf16(np.asarray(x[lo:hi], np.float32).T),
            "wts": wts, "bias": bias, "dinv_c": dcol,
            "iota": iota_t, "identb": ident_b,
            "idx": idx, "doff": doff,
        })
    t2 = time.time()
    kw = {}
    if trace and os.environ.get("GCN_TRACEDIR"):
        kw["tmpdir"] = os.environ["GCN_TRACEDIR"]
    res = run_bass_kernel_spmd(nc, in_maps, core_ids=list(range(NCORES)),
                               trace=trace, **kw)
    if os.environ.get("GCN_VERBOSE"):
        print(f"[kernel] prep inputs: {t2 - t1:.1f}s, "
              f"run: {time.time() - t2:.1f}s", flush=True)
    out = np.concatenate([res.results[c]["out"] for c in range(NCORES)],
                         axis=0)
    return out, res


def kernel(x, edge_index, W1, b1, W2, b2):
    plan = Plan(x.shape[0], np.asarray(edge_index))
    out, _ = _run(plan, np.asarray(x), np.asarray(W1), np.asarray(b1),
                  np.asarray(W2), np.asarray(b2))
    return out


# revision 21
# speedup vs baseline: 2.0688x; 2.0688x over previous
"""Trainium2 Bass kernel: 2-layer GCN (PyG-style GCNConv x2) on 8 NeuronCores.

Strategy (v2, src-sharded):
  - Nodes sharded contiguously across 8 cores (12500 rows each); edges are
    processed on the core that OWNS THE SOURCE node, so the per-layer gather
    reads only the core's own 12500-row table (kept in both SBUF and DRAM,
    bf16).
  - Per layer: dense h' = (x @ W) * dinv[src] on the owning core; the edge
    stream (sorted by dst) is gathered in 2048-slot super-blocks split
    between DRAM-source dma_gather (1024 idx, slot-major) and SBUF-source
    transpose dma_gather (2x512 idx, feat-major + PE transpose), spread over
    4 SWDGE queues so descriptor generation and transfers overlap.
  - Scatter-add via one-hot matmuls (bf16) into PSUM accumulators, one
    [128, 2, 128] accumulator per 256-dst-node "pair" segment; partial sums
    for all 100K dst nodes are evicted bf16 and combined with a
    ReduceScatter(add), which lands each core's own 12500 rows.
  - dinv[dst] scaling + bias are applied after the ReduceScatter on the
    owner core; layer 2 repeats the pipeline on the layer-1 output.
"""

import os
import sys

for _p in ("/opt/trn_rl_repo",):
    if _p not in sys.path:
        sys.path.append(_p)

import numpy as np

import concourse.bacc as bacc
import concourse.mybir as mybir
import concourse.tile as tile
from concourse.bass_utils import run_bass_kernel_spmd

F32 = mybir.dt.float32
BF16 = mybir.dt.bfloat16
I16 = mybir.dt.int16
AF = mybir.ActivationFunctionType
ALU = mybir.AluOpType

N_NODES = 100000
D = 128
NCORES = 8
NS = N_NODES // NCORES           # 12500
NTILE = -(-NS // 128)            # 98
NPAIR = -(-NS // 256)            # 49
NPID = NCORES * NPAIR            # 392
SB = 2048                        # slots per super-block
SBCH = SB // 128                 # 16 chunks per super-block


def _bf16(a):
    try:
        import ml_dtypes
        return np.asarray(a, dtype=ml_dtypes.bfloat16)
    except ImportError:
        import jax.numpy as jnp
        return np.asarray(jnp.asarray(a, dtype=jnp.bfloat16))


class Plan:
    """Core-uniform gather/matmul schedule derived from the edge index."""

    def __init__(self, n_nodes, edge_index):
        assert n_nodes == N_NODES
        e = np.asarray(edge_index)
        # self-loops are excluded from the gather stream: their contribution
        # dinv[i]*h'[i] is local to the owner and is added in post().
        src = np.asarray(e[0], np.int64)
        dst = np.asarray(e[1], np.int64)
        deg = (np.bincount(dst, minlength=n_nodes) + 1).astype(np.float32)
        self.dinv = deg ** -0.5

        owner = src // NS
        order = np.argsort(owner * n_nodes + dst, kind="stable")
        srcl = (src - owner * NS)[order].astype(np.int16)
        dsts = dst[order]
        own = owner[order].astype(np.int64)
        r = dsts % NS
        pid = (dsts // NS) * NPAIR + r // 256
        sub = (r // 128) % 2
        doffv = (r % 128).astype(np.float32)

        E = len(srcl)
        cnt = np.bincount(own * NPID + pid, minlength=NCORES * NPID
                          ).reshape(NCORES, NPID)
        CT = np.maximum(-(-cnt.max(axis=0) // 128), 1)
        self.CT = CT
        CTmax = int(CT.max())
        sbase = np.zeros(NPID + 1, np.int64)
        np.cumsum(CT * 128, out=sbase[1:])
        raw_slots = int(sbase[-1])
        self.nsb = -(-raw_slots // SB)
        self.n_slots = self.nsb * SB
        nch = self.n_slots // 128

        segkey = own * NPID + pid
        changed = np.r_[True, segkey[1:] != segkey[:-1]]
        seg_first = np.flatnonzero(changed)
        rank = np.arange(E) - seg_first[np.cumsum(changed) - 1]
        jch = rank // 128
        sic = rank % 128
        abs_slot = sbase[pid] + rank

        self._idx_stream = np.zeros((NCORES, self.n_slots), np.int16)
        self._idx_stream[own, abs_slot] = srcl

        ekey = (pid * CTmax + jch) * 2 + sub
        emit = np.bincount(ekey, minlength=NPID * CTmax * 2) > 0
        emit = emit.reshape(NPID, CTmax, 2)
        jmask = np.arange(CTmax)[None, :] < CT[:, None]
        emit &= jmask[:, :, None]
        # every pair always has both subtiles (self-loops); ensure at least
        # one column per (pid, s) so every acc region is written
        for s in (0, 1):
            none = ~emit[:, :, s].any(axis=1)
            emit[none, 0, s] = True
        col_of = np.full((NPID, CTmax, 2), -1, np.int64)
        ncols = int(emit.sum())
        # column ids in (pid, j, s) lexicographic order == stream order
        flat = emit.reshape(-1)
        col_of.reshape(-1)[flat] = np.arange(ncols)
        self.ncols = ncols

        doff_t = np.full((NCORES, 128, ncols), -1.0, np.float32)
        colid = col_of[pid, jch, sub]
        assert (colid >= 0).all()
        doff_t[own, sic, colid] = doffv
        self._doff = doff_t

        pid_of_chunk = np.repeat(np.arange(NPID), CT)
        j_of_chunk = np.concatenate([np.arange(c) for c in CT])
        first_col = np.full((NPID, 2), -1, np.int64)
        last_col = np.full((NPID, 2), -1, np.int64)
        for s in (0, 1):
            for p in range(NPID):
                js = np.flatnonzero(emit[p, :, s])
                first_col[p, s] = col_of[p, js[0], s]
                last_col[p, s] = col_of[p, js[-1], s]
        self.raw_chunks = len(pid_of_chunk)
        self.mm_sched = [[] for _ in range(nch)]
        for k in range(self.raw_chunks):
            p, j = int(pid_of_chunk[k]), int(j_of_chunk[k])
            for s in (0, 1):
                c = int(col_of[p, j, s])
                if c >= 0:
                    self.mm_sched[k].append(
                        (c, p, s, c == first_col[p, s], c == last_col[p, s]))
        self.ev_after = [[] for _ in range(nch)]
        for p in range(NPID):
            k_last = int(sbase[p] // 128 + CT[p] - 1)
            self.ev_after[k_last].append(p)

        self.sb_cols = []
        for sbi in range(self.nsb):
            k0 = sbi * SBCH
            k1 = min((sbi + 1) * SBCH, self.raw_chunks)
            cols = [c for k in range(k0, k1) for (c, *_r) in self.mm_sched[k]]
            self.sb_cols.append((min(cols), max(cols) + 1) if cols else (0, 0))
        self.max_sb_cols = max(c1 - c0 for c0, c1 in self.sb_cols)

        self.call_live = []
        for sbi in range(self.nsb):
            base = sbi * SB
            self.call_live.append((base < raw_slots,
                                   base + 1024 < raw_slots,
                                   base + 1536 < raw_slots))
        self.raw_slots = raw_slots
        self.icols = self.nsb * 128

    def core_inputs(self, c):
        st = self._idx_stream[c].reshape(self.nsb, SB)
        idx = np.zeros((16, self.nsb, 128), np.int16)
        idx[:, :, 0:64] = st[:, :1024].reshape(self.nsb, 64, 16
                                               ).transpose(2, 0, 1)
        idx[:, :, 64:96] = st[:, 1024:1536].reshape(self.nsb, 32, 16
                                                    ).transpose(2, 0, 1)
        idx[:, :, 96:128] = st[:, 1536:2048].reshape(self.nsb, 32, 16
                                                     ).transpose(2, 0, 1)
        idx = idx.reshape(16, self.icols)
        return np.tile(idx, (8, 1)), self._doff[c]


def _build(plan):
    nc = bacc.Bacc("TRN2", target_bir_lowering=False, debug=False,
                   num_devices=NCORES, num_swdge_queues=4)
    xT_d = nc.dram_tensor("xT", [D, NS], BF16, kind="ExternalInput").ap()
    wts_d = nc.dram_tensor("wts", [D, 2, D], BF16, kind="ExternalInput").ap()
    bias_d = nc.dram_tensor("bias", [D, 2, D], F32, kind="ExternalInput").ap()
    dv_d = nc.dram_tensor("dinv_c", [D, NTILE], F32, kind="ExternalInput").ap()
    iota_d = nc.dram_tensor("iota", [D, D], F32, kind="ExternalInput").ap()
    idn_d = nc.dram_tensor("identb", [D, D], BF16, kind="ExternalInput").ap()
    idx_d = nc.dram_tensor("idx", [D, plan.icols], I16,
                           kind="ExternalInput").ap()
    doff_d = nc.dram_tensor("doff", [D, plan.ncols], F32,
                            kind="ExternalInput").ap()
    out_d = nc.dram_tensor("out", [NS, D], F32, kind="ExternalOutput").ap()

    dbg = os.environ.get("GCN_DEBUG", "")
    def _k(nm):
        return "ExternalOutput" if nm in dbg.split(",") else "Internal"
    tbl_d = [nc.dram_tensor(f"tbl{i}", [NTILE * 128, D], BF16,
                            kind=_k(f"tbl{i}")).ap() for i in range(2)]
    part_d = [nc.dram_tensor(f"part{i}", [N_NODES, D], BF16,
                             kind=_k(f"part{i}")).ap() for i in range(2)]
    shard_d = [nc.dram_tensor(f"shard{i}", [NS, D], BF16,
                              kind=_k(f"shard{i}")).ap() for i in range(2)]

    with tile.TileContext(nc) as tc:
        with (
            tc.tile_pool(name="const", bufs=1) as cpool,
            tc.tile_pool(name="stg", bufs=int(os.environ.get("GCN_BSTG","6"))) as stgpool,
            tc.tile_pool(name="gg", bufs=int(os.environ.get("GCN_BSTG","6"))) as gpool,
            tc.tile_pool(name="tt", bufs=int(os.environ.get("GCN_BSTG","6"))) as tpool,
            tc.tile_pool(name="oh", bufs=int(os.environ.get("GCN_BOH","4"))) as ohpool,
            tc.tile_pool(name="ev", bufs=int(os.environ.get("GCN_BEV","6"))) as evpool,
            tc.tile_pool(name="ld", bufs=4) as ldpool,
            tc.tile_pool(name="acc", bufs=4, space="PSUM") as accpool,
            tc.tile_pool(name="tr", bufs=2, space="PSUM") as trpool,
            tc.tile_pool(name="pp", bufs=1, space="PSUM") as pppool,
        ):
            w_sb = cpool.tile([D, 2, D], BF16, tag="w")
            nc.sync.dma_start(out=w_sb[:], in_=wts_d[:])
            bias_sb = cpool.tile([D, 2, D], F32, tag="bias")
            nc.sync.dma_start(out=bias_sb[:], in_=bias_d[:])
            dv_sb = cpool.tile([D, NTILE], F32, tag="dv")
            nc.sync.dma_start(out=dv_sb[:], in_=dv_d[:])
            iota_sb = cpool.tile([D, D], F32, tag="iota")
            nc.sync.dma_start(out=iota_sb[:], in_=iota_d[:])
            idn_sb = cpool.tile([D, D], BF16, tag="idn")
            nc.sync.dma_start(out=idn_sb[:], in_=idn_d[:])
            idx_sb = cpool.tile([D, plan.icols], I16, tag="idx")
            nc.sync.dma_start(out=idx_sb[:], in_=idx_d[:])
            doff_sb = cpool.tile([D, plan.ncols], F32, tag="doff")
            nc.sync.dma_start(out=doff_sb[:], in_=doff_d[:])
            xT_sb = cpool.tile([D, NS], BF16, tag="xT")
            nc.sync.dma_start(out=xT_sb[:], in_=xT_d[:])
            xT2_sb = cpool.tile([D, NS], BF16, tag="xT2")
            tbl_sb = [cpool.tile([D, NTILE, D], BF16, tag=f"tblsb{i}",
                                 name=f"tblsb{i}")
                      for i in range(2)]

            def tw(t):
                return 128 if t < NTILE - 1 else NS - 128 * (NTILE - 1)

            def dense(li, src_sb):
                # last stripe is only partially covered by rows; zero it so
                # the full-tile SBUF gather source is fully initialized
                nc.vector.memset(tbl_sb[li][:, NTILE - 1, :], 0.0)
                for t in range(NTILE):
                    w = tw(t)
                    pdt = pppool.tile([128, D], F32, tag="pd")
                    nc.tensor.matmul(pdt[:w, :],
                                     lhsT=src_sb[:, t * 128:t * 128 + w],
                                     rhs=w_sb[:, li, :], start=True, stop=True)
                    nc.scalar.activation(tbl_sb[li][:w, t, :], pdt[:w, :],
                                         AF.Copy, scale=dv_sb[:w, t:t + 1])
                    nc.sync.dma_start(out=tbl_d[li][t * 128:(t + 1) * 128, :],
                                      in_=tbl_sb[li][:, t, :])

            qctr = [0]

            def sparse(li):
                acc_open = {}
                for sbi in range(plan.nsb):
                    dlive, s0live, s1live = plan.call_live[sbi]
                    ibase = sbi * 128
                    stg = None
                    tts = [None, None]
                    if dlive:
                        stg = stgpool.tile([128, 8, D], BF16, tag="stg")
                        nc.gpsimd.dma_gather(stg[:], tbl_d[li][:],
                                             idx_sb[:, ibase:ibase + 64],
                                             1024, 1024, D,
                                             queue_num=qctr[0] % 4)
                        qctr[0] += 1
                    for m, live in enumerate((s0live, s1live)):
                        if not live:
                            continue
                        g = gpool.tile([128, 1, 512], BF16, tag="g")
                        nc.gpsimd.dma_gather(
                            g[:], tbl_sb[li][:],
                            idx_sb[:, ibase + 64 + 32 * m:
                                   ibase + 96 + 32 * m],
                            512, 512, D, transpose=True,
                            sbuf_tokens_per_rank=128,
                            sbuf_free_dim_per_rank=2 * D,
                            sbuf_free_dim_pad_per_rank=0,
                            sbuf_byte_offset=0,
                            queue_num=qctr[0] % 4)
                        qctr[0] += 1
                        tp = trpool.tile([128, 4, D], BF16, tag="tp")
                        for cc in range(4):
                            nc.tensor.transpose(tp[:, cc, :],
                                                g[:, 0, cc * 128:
                                                  (cc + 1) * 128],
                                                idn_sb[:])
                        tb = tpool.tile([128, 4, D], BF16, tag="tb")
                        nc.vector.tensor_copy(tb[:], tp[:])
                        tts[m] = tb
                    c0, c1 = plan.sb_cols[sbi]
                    oh = None
                    if c1 > c0:
                        oh = ohpool.tile([128, plan.max_sb_cols, D], BF16,
                                         tag="oh")
                        nc.vector.scalar_tensor_tensor(
                            out=oh[:, :c1 - c0, :],
                            in0=doff_sb[:, c0:c1].unsqueeze(2)
                                .broadcast_to([128, c1 - c0, D]),
                            scalar=1.0,
                            in1=iota_sb[:].unsqueeze(1)
                                .broadcast_to([128, c1 - c0, D]),
                            op0=ALU.mult, op1=ALU.is_equal)
                    k1 = min((sbi + 1) * SBCH, plan.raw_chunks)
                    for k in range(sbi * SBCH, k1):
                        j = k - sbi * SBCH
                        if j < 8:
                            rhs = stg[:, j, :]
                        else:
                            rhs = tts[(j - 8) // 4][:, (j - 8) % 4, :]
                        for (c, p, s, st, sp) in plan.mm_sched[k]:
                            if (p, s) not in acc_open:
                                acc_open[(p, s)] = accpool.tile(
                                    [128, D], F32, tag="acc",
                                    name=f"a{li}_{p}_{s}")
                            nc.tensor.matmul(acc_open[(p, s)][:],
                                             lhsT=oh[:, c - c0, :], rhs=rhs,
                                             start=st, stop=sp)
                        for p in plan.ev_after[k]:
                            dc, pl = divmod(p, NPAIR)
                            r0 = dc * NS + pl * 256
                            for s in (0, 1):
                                acc = acc_open.pop((p, s))
                                w = min(128, NS - pl * 256 - s * 128)
                                if w <= 0:
                                    continue
                                ev = evpool.tile([128, D], BF16, tag="ev")
                                nc.scalar.activation(ev[:w, :], acc[:w, :],
                                                     AF.Copy)
                                nc.sync.dma_start(
                                    out=part_d[li][r0 + s * 128:
                                                   r0 + s * 128 + w, :],
                                    in_=ev[:w, :])
                assert not acc_open
                nc.gpsimd.collective_compute(
                    "ReduceScatter", ALU.add,
                    replica_groups=[list(range(NCORES))],
                    ins=[part_d[li].opt()], outs=[shard_d[li].opt()])

            def post(li):
                for t in range(NTILE):
                    w = tw(t)
                    sld = ldpool.tile([128, D], BF16, tag="sld")
                    nc.sync.dma_start(out=sld[:w, :],
                                      in_=shard_d[li][t * 128:t * 128 + w, :])
                    # add the self-loop term h'[i] (local table row), then
                    # scale by dinv[dst]
                    ys = ldpool.tile([128, D], F32, tag="ys")
                    nc.vector.tensor_tensor(out=ys[:w, :], in0=sld[:w, :],
                                            in1=tbl_sb[li][:w, t, :],
                                            op=ALU.add)
                    ya = ldpool.tile([128, D], F32, tag="ya")
                    nc.scalar.activation(ya[:w, :], ys[:w, :], AF.Copy,
                                         scale=dv_sb[:w, t:t + 1])
                    if li == 0:
                        yb = ldpool.tile([128, D], BF16, tag="yb")
                        nc.vector.tensor_tensor(out=yb[:w, :], in0=ya[:w, :],
                                                in1=bias_sb[:w, li, :],
                                                op=ALU.add)
                        ptr = pppool.tile([D, 128], BF16, tag="ptr")
                        nc.tensor.transpose(ptr[:, :w], yb[:w, :],
                                            idn_sb[:w, :w])
                        nc.vector.tensor_copy(xT2_sb[:, t * 128:t * 128 + w],
                                              ptr[:, :w])
                    else:
                        yb = ldpool.tile([128, D], F32, tag="ybf")
                        nc.vector.tensor_tensor(out=yb[:w, :], in0=ya[:w, :],
                                                in1=bias_sb[:w, li, :],
                                                op=ALU.add)
                        nc.sync.dma_start(out=out_d[t * 128:t * 128 + w, :],
                                          in_=yb[:w, :])

            dense(0, xT_sb)
            sparse(0)
            post(0)
            dense(1, xT2_sb)
            sparse(1)
            post(1)

    # The tile scheduler assigns DMASW completion lanes round-robin over the
    # *scheduled* SWDGE order, and each lane must stay bound to one SWDGE
    # queue. Rewrite queue_num := lane % 4 after scheduling so the binding is
    # consistent by construction.
    from concourse.tile_sem_assignment import PROC_NAME_TO_IDX
    lane_of = {PROC_NAME_TO_IDX[f"DMASW{i}"]: i for i in range(8)}
    for bb in nc.main_func.blocks:
        for inst in bb.instructions:
            proc = getattr(inst, "bass_scheduled_proc", None)
            if proc in lane_of and hasattr(inst, "queue_num"):
                inst.queue_num = lane_of[proc] % int(os.environ.get('GCN_NQ', '4'))

    nc.compile()
    return nc


def _install_ntff_hook():
    """antenv.axon_hooks is absent in this image; synthesize it and register
    the ctypes NTFF profile hook from the boot module."""
    import types
    if "antenv.axon_hooks" in sys.modules:
        return
    try:
        from trn_agent_boot.trn_boot import _ntff_profile_via_ctypes
        hook = _ntff_profile_via_ctypes("/opt/axon/libaxon_pjrt.so")
    except Exception as e:
        print(f"[kernel] ntff hook unavailable: {e}", flush=True)
        hook = None
    mod = types.ModuleType("antenv.axon_hooks")
    mod._hook = hook
    mod.set_axon_ntff_profile_hook = lambda h: setattr(mod, "_hook", h)
    mod.get_axon_ntff_profile_hook = lambda: mod._hook
    sys.modules["antenv.axon_hooks"] = mod
    import antenv
    antenv.axon_hooks = mod


def _run(plan, x, W1, b1, W2, b2, trace=False, stage="full"):
    import time
    if trace:
        _install_ntff_hook()
    t0 = time.time()
    nc = _build(plan)
    t1 = time.time()
    if os.environ.get("GCN_VERBOSE"):
        print(f"[kernel] build+compile: {t1 - t0:.1f}s", flush=True)

    wts = _bf16(np.stack([np.asarray(W1, np.float32),
                          np.asarray(W2, np.float32)], axis=1))
    bias = np.broadcast_to(
        np.stack([np.asarray(b1, np.float32), np.asarray(b2, np.float32)],
                 axis=0)[None, :, :], (D, 2, D)).copy()
    iota_t = np.tile(np.arange(D, dtype=np.float32), (D, 1))
    ident_b = _bf16(np.eye(D, dtype=np.float32))

    in_maps = []
    for c in range(NCORES):
        lo, hi = c * NS, (c + 1) * NS
        dcol = np.ones((NTILE, 128), dtype=np.float32)
        dcol.reshape(-1)[:NS] = plan.dinv[lo:hi]
        dcol = np.ascontiguousarray(dcol.T)
        idx, doff = plan.core_inputs(c)
        in_maps.append({
            "xT": _bf16(np.asarray(x[lo:hi], np.float32).T),
            "wts": wts, "bias": bias, "dinv_c": dcol,
            "iota": iota_t, "identb": ident_b,
            "idx": idx, "doff": doff,
        })
    t2 = time.time()
    kw = {}
    if trace and os.environ.get("GCN_TRACEDIR"):
        kw["tmpdir"] = os.environ["GCN_TRACEDIR"]
    res = run_bass_kernel_spmd(nc, in_maps, core_ids=list(range(NCORES)),
                               trace=trace, **kw)
    if os.environ.get("GCN_VERBOSE"):
        print(f"[kernel] prep inputs: {t2 - t1:.1f}s, "
              f"run: {time.time() - t2:.1f}s", flush=True)
    out = np.concatenate([res.results[c]["out"] for c in range(NCORES)],
                         axis=0)
    return out, res


def kernel(x, edge_index, W1, b1, W2, b2):
    plan = Plan(x.shape[0], np.asarray(edge_index))
    out, _ = _run(plan, np.asarray(x), np.asarray(W1), np.asarray(b1),
                  np.asarray(W2), np.asarray(b2))
    return out


# revision 22
# speedup vs baseline: 2.1097x; 1.0197x over previous
"""Trainium2 Bass kernel: 2-layer GCN (PyG-style GCNConv x2) on 8 NeuronCores.

Strategy (v2, src-sharded):
  - Nodes sharded contiguously across 8 cores (12500 rows each); edges are
    processed on the core that OWNS THE SOURCE node, so the per-layer gather
    reads only the core's own 12500-row table (kept in both SBUF and DRAM,
    bf16).
  - Per layer: dense h' = (x @ W) * dinv[src] on the owning core; the edge
    stream (sorted by dst) is gathered in 2048-slot super-blocks split
    between DRAM-source dma_gather (1024 idx, slot-major) and SBUF-source
    transpose dma_gather (2x512 idx, feat-major + PE transpose), spread over
    4 SWDGE queues so descriptor generation and transfers overlap.
  - Scatter-add via one-hot matmuls (bf16) into PSUM accumulators, one
    [128, 2, 128] accumulator per 256-dst-node "pair" segment; partial sums
    for all 100K dst nodes are evicted bf16 and combined with a
    ReduceScatter(add), which lands each core's own 12500 rows.
  - dinv[dst] scaling + bias are applied after the ReduceScatter on the
    owner core; layer 2 repeats the pipeline on the layer-1 output.
"""

import os
import sys

for _p in ("/opt/trn_rl_repo",):
    if _p not in sys.path:
        sys.path.append(_p)

import numpy as np

import concourse.bacc as bacc
import concourse.mybir as mybir
import concourse.tile as tile
from concourse.bass_utils import run_bass_kernel_spmd

F32 = mybir.dt.float32
BF16 = mybir.dt.bfloat16
I16 = mybir.dt.int16
AF = mybir.ActivationFunctionType
ALU = mybir.AluOpType

N_NODES = 100000
D = 128
NCORES = 8
NS = N_NODES // NCORES           # 12500
NTILE = -(-NS // 128)            # 98
NPAIR = -(-NS // 256)            # 49
NPID = NCORES * NPAIR            # 392
SB = 2048                        # slots per super-block
SBCH = SB // 128                 # 16 chunks per super-block
ALLSBUF = bool(int(os.environ.get("GCN_ALLSBUF", "1")))


def _bf16(a):
    try:
        import ml_dtypes
        return np.asarray(a, dtype=ml_dtypes.bfloat16)
    except ImportError:
        import jax.numpy as jnp
        return np.asarray(jnp.asarray(a, dtype=jnp.bfloat16))


class Plan:
    """Core-uniform gather/matmul schedule derived from the edge index."""

    def __init__(self, n_nodes, edge_index):
        assert n_nodes == N_NODES
        e = np.asarray(edge_index)
        # self-loops are excluded from the gather stream: their contribution
        # dinv[i]*h'[i] is local to the owner and is added in post().
        src = np.asarray(e[0], np.int64)
        dst = np.asarray(e[1], np.int64)
        deg = (np.bincount(dst, minlength=n_nodes) + 1).astype(np.float32)
        self.dinv = deg ** -0.5

        owner = src // NS
        order = np.argsort(owner * n_nodes + dst, kind="stable")
        srcl = (src - owner * NS)[order].astype(np.int16)
        dsts = dst[order]
        own = owner[order].astype(np.int64)
        r = dsts % NS
        pid = (dsts // NS) * NPAIR + r // 256
        sub = (r // 128) % 2
        doffv = (r % 128).astype(np.float32)

        E = len(srcl)
        cnt = np.bincount(own * NPID + pid, minlength=NCORES * NPID
                          ).reshape(NCORES, NPID)
        CT = np.maximum(-(-cnt.max(axis=0) // 128), 1)
        self.CT = CT
        CTmax = int(CT.max())
        sbase = np.zeros(NPID + 1, np.int64)
        np.cumsum(CT * 128, out=sbase[1:])
        raw_slots = int(sbase[-1])
        self.nsb = -(-raw_slots // SB)
        self.n_slots = self.nsb * SB
        nch = self.n_slots // 128

        segkey = own * NPID + pid
        changed = np.r_[True, segkey[1:] != segkey[:-1]]
        seg_first = np.flatnonzero(changed)
        rank = np.arange(E) - seg_first[np.cumsum(changed) - 1]
        jch = rank // 128
        sic = rank % 128
        abs_slot = sbase[pid] + rank

        self._idx_stream = np.zeros((NCORES, self.n_slots), np.int16)
        self._idx_stream[own, abs_slot] = srcl

        ekey = (pid * CTmax + jch) * 2 + sub
        emit = np.bincount(ekey, minlength=NPID * CTmax * 2) > 0
        emit = emit.reshape(NPID, CTmax, 2)
        jmask = np.arange(CTmax)[None, :] < CT[:, None]
        emit &= jmask[:, :, None]
        # every pair always has both subtiles (self-loops); ensure at least
        # one column per (pid, s) so every acc region is written
        for s in (0, 1):
            none = ~emit[:, :, s].any(axis=1)
            emit[none, 0, s] = True
        col_of = np.full((NPID, CTmax, 2), -1, np.int64)
        ncols = int(emit.sum())
        # column ids in (pid, j, s) lexicographic order == stream order
        flat = emit.reshape(-1)
        col_of.reshape(-1)[flat] = np.arange(ncols)
        self.ncols = ncols

        doff_t = np.full((NCORES, 128, ncols), -1.0, np.float32)
        colid = col_of[pid, jch, sub]
        assert (colid >= 0).all()
        doff_t[own, sic, colid] = doffv
        self._doff = doff_t

        pid_of_chunk = np.repeat(np.arange(NPID), CT)
        j_of_chunk = np.concatenate([np.arange(c) for c in CT])
        first_col = np.full((NPID, 2), -1, np.int64)
        last_col = np.full((NPID, 2), -1, np.int64)
        for s in (0, 1):
            for p in range(NPID):
                js = np.flatnonzero(emit[p, :, s])
                first_col[p, s] = col_of[p, js[0], s]
                last_col[p, s] = col_of[p, js[-1], s]
        self.raw_chunks = len(pid_of_chunk)
        self.mm_sched = [[] for _ in range(nch)]
        for k in range(self.raw_chunks):
            p, j = int(pid_of_chunk[k]), int(j_of_chunk[k])
            for s in (0, 1):
                c = int(col_of[p, j, s])
                if c >= 0:
                    self.mm_sched[k].append(
                        (c, p, s, c == first_col[p, s], c == last_col[p, s]))
        self.ev_after = [[] for _ in range(nch)]
        for p in range(NPID):
            k_last = int(sbase[p] // 128 + CT[p] - 1)
            self.ev_after[k_last].append(p)

        self.sb_cols = []
        for sbi in range(self.nsb):
            k0 = sbi * SBCH
            k1 = min((sbi + 1) * SBCH, self.raw_chunks)
            cols = [c for k in range(k0, k1) for (c, *_r) in self.mm_sched[k]]
            self.sb_cols.append((min(cols), max(cols) + 1) if cols else (0, 0))
        self.max_sb_cols = max(c1 - c0 for c0, c1 in self.sb_cols)

        self.call_live = []
        for sbi in range(self.nsb):
            base = sbi * SB
            if ALLSBUF:
                self.call_live.append(tuple(
                    base + 512 * m < raw_slots for m in range(4)))
            else:
                self.call_live.append((base < raw_slots,
                                       base + 1024 < raw_slots,
                                       base + 1536 < raw_slots))
        self.raw_slots = raw_slots
        self.icols = self.nsb * 128

    def core_inputs(self, c):
        st = self._idx_stream[c].reshape(self.nsb, SB)
        idx = np.zeros((16, self.nsb, 128), np.int16)
        if ALLSBUF:
            for m in range(4):
                idx[:, :, 32 * m:32 * (m + 1)] = (
                    st[:, 512 * m:512 * (m + 1)]
                    .reshape(self.nsb, 32, 16).transpose(2, 0, 1))
        else:
            idx[:, :, 0:64] = st[:, :1024].reshape(self.nsb, 64, 16
                                                   ).transpose(2, 0, 1)
            idx[:, :, 64:96] = st[:, 1024:1536].reshape(self.nsb, 32, 16
                                                        ).transpose(2, 0, 1)
            idx[:, :, 96:128] = st[:, 1536:2048].reshape(self.nsb, 32, 16
                                                         ).transpose(2, 0, 1)
        idx = idx.reshape(16, self.icols)
        return np.tile(idx, (8, 1)), self._doff[c]


def _build(plan):
    nc = bacc.Bacc("TRN2", target_bir_lowering=False, debug=False,
                   num_devices=NCORES, num_swdge_queues=4)
    xT_d = nc.dram_tensor("xT", [D, NS], BF16, kind="ExternalInput").ap()
    wts_d = nc.dram_tensor("wts", [D, 2, D], BF16, kind="ExternalInput").ap()
    bias_d = nc.dram_tensor("bias", [D, 2, D], F32, kind="ExternalInput").ap()
    dv_d = nc.dram_tensor("dinv_c", [D, NTILE], F32, kind="ExternalInput").ap()
    iota_d = nc.dram_tensor("iota", [D, D], F32, kind="ExternalInput").ap()
    idn_d = nc.dram_tensor("identb", [D, D], BF16, kind="ExternalInput").ap()
    idx_d = nc.dram_tensor("idx", [D, plan.icols], I16,
                           kind="ExternalInput").ap()
    doff_d = nc.dram_tensor("doff", [D, plan.ncols], F32,
                            kind="ExternalInput").ap()
    out_d = nc.dram_tensor("out", [NS, D], F32, kind="ExternalOutput").ap()

    dbg = os.environ.get("GCN_DEBUG", "")
    def _k(nm):
        return "ExternalOutput" if nm in dbg.split(",") else "Internal"
    tbl_d = [nc.dram_tensor(f"tbl{i}", [NTILE * 128, D], BF16,
                            kind=_k(f"tbl{i}")).ap() for i in range(2)]
    part_d = [nc.dram_tensor(f"part{i}", [N_NODES, D], BF16,
                             kind=_k(f"part{i}")).ap() for i in range(2)]
    shard_d = [nc.dram_tensor(f"shard{i}", [NS, D], BF16,
                              kind=_k(f"shard{i}")).ap() for i in range(2)]

    with tile.TileContext(nc) as tc:
        with (
            tc.tile_pool(name="const", bufs=1) as cpool,
            tc.tile_pool(name="stg", bufs=int(os.environ.get("GCN_BSTG","6"))) as stgpool,
            tc.tile_pool(name="gg", bufs=int(os.environ.get("GCN_BSTG","6"))) as gpool,
            tc.tile_pool(name="tt", bufs=int(os.environ.get("GCN_BSTG","6"))) as tpool,
            tc.tile_pool(name="oh", bufs=int(os.environ.get("GCN_BOH","4"))) as ohpool,
            tc.tile_pool(name="ev", bufs=int(os.environ.get("GCN_BEV","6"))) as evpool,
            tc.tile_pool(name="ld", bufs=4) as ldpool,
            tc.tile_pool(name="acc", bufs=4, space="PSUM") as accpool,
            tc.tile_pool(name="tr", bufs=2, space="PSUM") as trpool,
            tc.tile_pool(name="pp", bufs=1, space="PSUM") as pppool,
        ):
            w_sb = cpool.tile([D, 2, D], BF16, tag="w")
            nc.sync.dma_start(out=w_sb[:], in_=wts_d[:])
            bias_sb = cpool.tile([D, 2, D], F32, tag="bias")
            nc.sync.dma_start(out=bias_sb[:], in_=bias_d[:])
            dv_sb = cpool.tile([D, NTILE], F32, tag="dv")
            nc.sync.dma_start(out=dv_sb[:], in_=dv_d[:])
            iota_sb = cpool.tile([D, D], F32, tag="iota")
            nc.sync.dma_start(out=iota_sb[:], in_=iota_d[:])
            idn_sb = cpool.tile([D, D], BF16, tag="idn")
            nc.sync.dma_start(out=idn_sb[:], in_=idn_d[:])
            idx_sb = cpool.tile([D, plan.icols], I16, tag="idx")
            nc.sync.dma_start(out=idx_sb[:], in_=idx_d[:])
            doff_sb = cpool.tile([D, plan.ncols], F32, tag="doff")
            nc.sync.dma_start(out=doff_sb[:], in_=doff_d[:])
            xT_sb = cpool.tile([D, NS], BF16, tag="xT")
            nc.sync.dma_start(out=xT_sb[:], in_=xT_d[:])
            xT2_sb = cpool.tile([D, NS], BF16, tag="xT2")
            tbl_sb = [cpool.tile([D, NTILE, D], BF16, tag=f"tblsb{i}",
                                 name=f"tblsb{i}")
                      for i in range(2)]

            def tw(t):
                return 128 if t < NTILE - 1 else NS - 128 * (NTILE - 1)

            def dense(li, src_sb):
                # last stripe is only partially covered by rows; zero it so
                # the full-tile SBUF gather source is fully initialized
                nc.vector.memset(tbl_sb[li][:, NTILE - 1, :], 0.0)
                for t in range(NTILE):
                    w = tw(t)
                    pdt = pppool.tile([128, D], F32, tag="pd")
                    nc.tensor.matmul(pdt[:w, :],
                                     lhsT=src_sb[:, t * 128:t * 128 + w],
                                     rhs=w_sb[:, li, :], start=True, stop=True)
                    nc.scalar.activation(tbl_sb[li][:w, t, :], pdt[:w, :],
                                         AF.Copy, scale=dv_sb[:w, t:t + 1])
                    if not ALLSBUF:
                        nc.sync.dma_start(
                            out=tbl_d[li][t * 128:(t + 1) * 128, :],
                            in_=tbl_sb[li][:, t, :])

            qctr = [0]

            def sparse(li):
                acc_open = {}
                for sbi in range(plan.nsb):
                    lives = plan.call_live[sbi]
                    ibase = sbi * 128
                    stg = None
                    tts = [None, None, None, None]
                    if ALLSBUF:
                        for m, live in enumerate(lives):
                            if not live:
                                continue
                            g = gpool.tile([128, 1, 512], BF16, tag="g")
                            nc.gpsimd.dma_gather(
                                g[:], tbl_sb[li][:],
                                idx_sb[:, ibase + 32 * m:
                                       ibase + 32 * (m + 1)],
                                512, 512, D, transpose=True,
                                sbuf_tokens_per_rank=128,
                                sbuf_free_dim_per_rank=2 * D,
                                sbuf_free_dim_pad_per_rank=0,
                                sbuf_byte_offset=0,
                                queue_num=qctr[0] % 4)
                            qctr[0] += 1
                            tp = trpool.tile([128, 4, D], BF16, tag="tp")
                            for cc in range(4):
                                nc.tensor.transpose(tp[:, cc, :],
                                                    g[:, 0, cc * 128:
                                                      (cc + 1) * 128],
                                                    idn_sb[:])
                            tb = tpool.tile([128, 4, D], BF16, tag="tb")
                            nc.vector.tensor_copy(tb[:], tp[:])
                            tts[m] = tb
                    else:
                        dlive, s0live, s1live = lives
                        if dlive:
                            stg = stgpool.tile([128, 8, D], BF16, tag="stg")
                            nc.gpsimd.dma_gather(stg[:], tbl_d[li][:],
                                                 idx_sb[:, ibase:ibase + 64],
                                                 1024, 1024, D,
                                                 queue_num=qctr[0] % 4)
                            qctr[0] += 1
                        for m, live in enumerate((s0live, s1live)):
                            if not live:
                                continue
                            g = gpool.tile([128, 1, 512], BF16, tag="g")
                            nc.gpsimd.dma_gather(
                                g[:], tbl_sb[li][:],
                                idx_sb[:, ibase + 64 + 32 * m:
                                       ibase + 96 + 32 * m],
                                512, 512, D, transpose=True,
                                sbuf_tokens_per_rank=128,
                                sbuf_free_dim_per_rank=0 + 2 * D,
                                sbuf_free_dim_pad_per_rank=0,
                                sbuf_byte_offset=0,
                                queue_num=qctr[0] % 4)
                            qctr[0] += 1
                            tp = trpool.tile([128, 4, D], BF16, tag="tp")
                            for cc in range(4):
                                nc.tensor.transpose(tp[:, cc, :],
                                                    g[:, 0, cc * 128:
                                                      (cc + 1) * 128],
                                                    idn_sb[:])
                            tb = tpool.tile([128, 4, D], BF16, tag="tb")
                            nc.vector.tensor_copy(tb[:], tp[:])
                            tts[m + 2] = tb
                    c0, c1 = plan.sb_cols[sbi]
                    oh = None
                    if c1 > c0:
                        oh = ohpool.tile([128, plan.max_sb_cols, D], BF16,
                                         tag="oh")
                        nc.vector.scalar_tensor_tensor(
                            out=oh[:, :c1 - c0, :],
                            in0=doff_sb[:, c0:c1].unsqueeze(2)
                                .broadcast_to([128, c1 - c0, D]),
                            scalar=1.0,
                            in1=iota_sb[:].unsqueeze(1)
                                .broadcast_to([128, c1 - c0, D]),
                            op0=ALU.mult, op1=ALU.is_equal)
                    k1 = min((sbi + 1) * SBCH, plan.raw_chunks)
                    for k in range(sbi * SBCH, k1):
                        j = k - sbi * SBCH
                        if ALLSBUF:
                            rhs = tts[j // 4][:, j % 4, :]
                        elif j < 8:
                            rhs = stg[:, j, :]
                        else:
                            rhs = tts[2 + (j - 8) // 4][:, (j - 8) % 4, :]
                        for (c, p, s, st, sp) in plan.mm_sched[k]:
                            if (p, s) not in acc_open:
                                acc_open[(p, s)] = accpool.tile(
                                    [128, D], F32, tag="acc",
                                    name=f"a{li}_{p}_{s}")
                            nc.tensor.matmul(acc_open[(p, s)][:],
                                             lhsT=oh[:, c - c0, :], rhs=rhs,
                                             start=st, stop=sp)
                        for p in plan.ev_after[k]:
                            dc, pl = divmod(p, NPAIR)
                            r0 = dc * NS + pl * 256
                            for s in (0, 1):
                                acc = acc_open.pop((p, s))
                                w = min(128, NS - pl * 256 - s * 128)
                                if w <= 0:
                                    continue
                                ev = evpool.tile([128, D], BF16, tag="ev")
                                nc.scalar.activation(ev[:w, :], acc[:w, :],
                                                     AF.Copy)
                                nc.sync.dma_start(
                                    out=part_d[li][r0 + s * 128:
                                                   r0 + s * 128 + w, :],
                                    in_=ev[:w, :])
                assert not acc_open
                nc.gpsimd.collective_compute(
                    "ReduceScatter", ALU.add,
                    replica_groups=[list(range(NCORES))],
                    ins=[part_d[li].opt()], outs=[shard_d[li].opt()])

            def post(li):
                for t in range(NTILE):
                    w = tw(t)
                    sld = ldpool.tile([128, D], BF16, tag="sld")
                    nc.sync.dma_start(out=sld[:w, :],
                                      in_=shard_d[li][t * 128:t * 128 + w, :])
                    # add the self-loop term h'[i] (local table row), then
                    # scale by dinv[dst]
                    ys = ldpool.tile([128, D], F32, tag="ys")
                    nc.vector.tensor_tensor(out=ys[:w, :], in0=sld[:w, :],
                                            in1=tbl_sb[li][:w, t, :],
                                            op=ALU.add)
                    ya = ldpool.tile([128, D], F32, tag="ya")
                    nc.scalar.activation(ya[:w, :], ys[:w, :], AF.Copy,
                                         scale=dv_sb[:w, t:t + 1])
                    if li == 0:
                        yb = ldpool.tile([128, D], BF16, tag="yb")
                        nc.vector.tensor_tensor(out=yb[:w, :], in0=ya[:w, :],
                                                in1=bias_sb[:w, li, :],
                                                op=ALU.add)
                        ptr = pppool.tile([D, 128], BF16, tag="ptr")
                        nc.tensor.transpose(ptr[:, :w], yb[:w, :],
                                            idn_sb[:w, :w])
                        nc.vector.tensor_copy(xT2_sb[:, t * 128:t * 128 + w],
                                              ptr[:, :w])
                    else:
                        yb = ldpool.tile([128, D], F32, tag="ybf")
                        nc.vector.tensor_tensor(out=yb[:w, :], in0=ya[:w, :],
                                                in1=bias_sb[:w, li, :],
                                                op=ALU.add)
                        nc.sync.dma_start(out=out_d[t * 128:t * 128 + w, :],
                                          in_=yb[:w, :])

            dense(0, xT_sb)
            sparse(0)
            post(0)
            dense(1, xT2_sb)
            sparse(1)
            post(1)

    # The tile scheduler assigns DMASW completion lanes round-robin over the
    # *scheduled* SWDGE order, and each lane must stay bound to one SWDGE
    # queue. Rewrite queue_num := lane % 4 after scheduling so the binding is
    # consistent by construction.
    from concourse.tile_sem_assignment import PROC_NAME_TO_IDX
    lane_of = {PROC_NAME_TO_IDX[f"DMASW{i}"]: i for i in range(8)}
    for bb in nc.main_func.blocks:
        for inst in bb.instructions:
            proc = getattr(inst, "bass_scheduled_proc", None)
            if proc in lane_of and hasattr(inst, "queue_num"):
                inst.queue_num = lane_of[proc] % int(os.environ.get('GCN_NQ', '4'))

    nc.compile()
    return nc


def _install_ntff_hook():
    """antenv.axon_hooks is absent in this image; synthesize it and register
    the ctypes NTFF profile hook from the boot module."""
    import types
    if "antenv.axon_hooks" in sys.modules:
        return
    try:
        from trn_agent_boot.trn_boot import _ntff_profile_via_ctypes
        hook = _ntff_profile_via_ctypes("/opt/axon/libaxon_pjrt.so")
    except Exception as e:
        print(f"[kernel] ntff hook unavailable: {e}", flush=True)
        hook = None
    mod = types.ModuleType("antenv.axon_hooks")
    mod._hook = hook
    mod.set_axon_ntff_profile_hook = lambda h: setattr(mod, "_hook", h)
    mod.get_axon_ntff_profile_hook = lambda: mod._hook
    sys.modules["antenv.axon_hooks"] = mod
    import antenv
    antenv.axon_hooks = mod


def _run(plan, x, W1, b1, W2, b2, trace=False, stage="full"):
    import time
    if trace:
        _install_ntff_hook()
    t0 = time.time()
    nc = _build(plan)
    t1 = time.time()
    if os.environ.get("GCN_VERBOSE"):
        print(f"[kernel] build+compile: {t1 - t0:.1f}s", flush=True)

    wts = _bf16(np.stack([np.asarray(W1, np.float32),
                          np.asarray(W2, np.float32)], axis=1))
    bias = np.broadcast_to(
        np.stack([np.asarray(b1, np.float32), np.asarray(b2, np.float32)],
                 axis=0)[None, :, :], (D, 2, D)).copy()
    iota_t = np.tile(np.arange(D, dtype=np.float32), (D, 1))
    ident_b = _bf16(np.eye(D, dtype=np.float32))

    in_maps = []
    for c in range(NCORES):
        lo, hi = c * NS, (c + 1) * NS
        dcol = np.ones((NTILE, 128), dtype=np.float32)
        dcol.reshape(-1)[:NS] = plan.dinv[lo:hi]
        dcol = np.ascontiguousarray(dcol.T)
        idx, doff = plan.core_inputs(c)
        in_maps.append({
            "xT": _bf16(np.asarray(x[lo:hi], np.float32).T),
            "wts": wts, "bias": bias, "dinv_c": dcol,
            "iota": iota_t, "identb": ident_b,
            "idx": idx, "doff": doff,
        })
    t2 = time.time()
    kw = {}
    if trace and os.environ.get("GCN_TRACEDIR"):
        kw["tmpdir"] = os.environ["GCN_TRACEDIR"]
    res = run_bass_kernel_spmd(nc, in_maps, core_ids=list(range(NCORES)),
                               trace=trace, **kw)
    if os.environ.get("GCN_VERBOSE"):
        print(f"[kernel] prep inputs: {t2 - t1:.1f}s, "
              f"run: {time.time() - t2:.1f}s", flush=True)
    out = np.concatenate([res.results[c]["out"] for c in range(NCORES)],
                         axis=0)
    return out, res


def kernel(x, edge_index, W1, b1, W2, b2):
    plan = Plan(x.shape[0], np.asarray(edge_index))
    out, _ = _run(plan, np.asarray(x), np.asarray(W1), np.asarray(b1),
                  np.asarray(W2), np.asarray(b2))
    return out
